# revision 3
# baseline (speedup 1.0000x reference)
"""Trainium2 Bass kernel for the 4-chain masked RNN (ArbitraryStructureRNN).

Structure: out = last step of (layer2 + skip), where
  layer0: x -> h0 (RNN tanh, W_ih0 unmasked)
  layer1: h0 -> h1 (masked W_ih1)
  layer2: h1 -> h2 (masked W_ih2)
  skip:   h0 -> hs (masked W_ihs), added to layer2 output at the end.

Sharding: data-parallel over batch (B=64 -> 8 cores x 8).
Per core all 4 chains run as a wavefront (chain c lags its input producer
by one 16-step group). Input projections for a 16-step group are matmul'd
into the PSUM bank first (sets has_written), the serial recurrence matmuls
then accumulate on top (start=False) so the per-step add is free; tanh is
a single strided ACT per chain-step reading PSUM and writing the bf16
h-history in SBUF, which feeds both the next step's matmuls and the
downstream chain's projections.
"""

import sys, types

for _p in ("/opt/trn_rl_repo",):
    if _p not in sys.path:
        sys.path.append(_p)

import numpy as np

# make run_bass_kernel_spmd(trace=True) usable under axon (optional here)
try:
    import antenv

    if not hasattr(antenv, "axon_hooks"):
        _hooks = types.ModuleType("antenv.axon_hooks")
        _h = [None]
        _hooks.set_axon_ntff_profile_hook = lambda h: _h.__setitem__(0, h)
        _hooks.get_axon_ntff_profile_hook = lambda: _h[0]
        sys.modules["antenv.axon_hooks"] = _hooks
        antenv.axon_hooks = _hooks
        try:
            from trn_agent_boot.trn_boot import _ntff_profile_via_ctypes

            _hooks.set_axon_ntff_profile_hook(
                _ntff_profile_via_ctypes("/opt/axon/libaxon_pjrt.so")
            )
        except Exception:
            pass
except Exception:
    pass

from concourse import bacc, tile
from concourse import bass_utils
from concourse.bass import mybir

BF16 = mybir.dt.bfloat16
F32 = mybir.dt.float32

H = 512
IN = 256
NCORES = 8
GS = 16  # steps per psum bank group

# chain input-feature k-tile counts: c0 reads x (256 = 2 tiles), others read h (4)
KIN = [2, 4, 4, 4]
WM_BASE = [0, 1024, 3072, 5120]  # col base of each chain in wmT
WM_COLS = WM_BASE[-1] + 2048


def _tileize(a):
    """[K, M] -> [128, (K/128)*(M/128)*128] with col = (k*nm + m)*128 + j."""
    K, M = a.shape
    nk, nm = K // 128, M // 128
    return (
        a.reshape(nk, 128, nm, 128).transpose(1, 0, 2, 3).reshape(128, nk * nm * 128)
    )


def build(T, b):
    NG = T // GS
    nc = bacc.Bacc("TRN2", target_bir_lowering=False, debug=False, num_devices=NCORES)
    xT_d = nc.dram_tensor("xT", [128, 2 * T * b], BF16, kind="ExternalInput").ap()
    wmT_d = nc.dram_tensor("wmT", [128, WM_COLS], BF16, kind="ExternalInput").ap()
    whT_d = nc.dram_tensor("whT", [128, 8192], BF16, kind="ExternalInput").ap()
    aux_d = nc.dram_tensor("aux", [1, 2560], BF16, kind="ExternalInput").ap()
    bT_d = nc.dram_tensor("biasT", [128, 2048], BF16, kind="ExternalInput").ap()
    out_d = nc.dram_tensor("out", [128, 4 * b], F32, kind="ExternalOutput").ap()

    with tile.TileContext(nc) as tc:
        with (
            tc.tile_pool(name="const", bufs=1) as cpool,
            tc.tile_pool(name="hist", bufs=1) as hpool,
            tc.tile_pool(name="ps", bufs=2, space="PSUM") as ppool,
        ):
            xT = cpool.tile([128, 2 * T * b], BF16, tag="xT")
            wmT = cpool.tile([128, WM_COLS], BF16, tag="wmT")
            whT = cpool.tile([128, 8192], BF16, tag="whT")
            aux = cpool.tile([1, 2560], BF16, tag="aux")
            bT = cpool.tile([128, 2048], BF16, tag="bT")
            # need-ordered, sliced input loads so the prologue/first groups
            # overlap the bulk DMA: wmT-c0 + first x slices + bias first,
            # then whT per chain in wavefront order, then the rest.
            def dmas(dst, src, cuts):
                for lo, hi in cuts:
                    nc.sync.dma_start(out=dst[:, lo:hi], in_=src[:, lo:hi])

            dmas(wmT, wmT_d, [(0, 1024)])  # c0 proj tiles (prologue)
            # xT group slices, earliest groups first (k-tile blocks of T*b)
            xg = []
            for gchunk in range(0, NG, 4):
                for k in range(2):
                    lo = k * T * b + gchunk * GS * b
                    xg.append((lo, lo + 4 * GS * b))
            dmas(xT, xT_d, xg[:2])
            dmas(bT, bT_d, [(0, 2048)])
            dmas(whT, whT_d, [(0, 2048)])  # c0 recurrence
            dmas(xT, xT_d, xg[2:6])
            dmas(wmT, wmT_d, [(1024, 3072), (5120, 7168)])  # c1, cs proj
            dmas(whT, whT_d, [(2048, 4096), (6144, 8192)])  # c1, cs rec
            dmas(wmT, wmT_d, [(3072, 5120)])  # c2 proj
            dmas(whT, whT_d, [(4096, 6144)])  # c2 rec
            dmas(xT, xT_d, xg[6:])
            nc.sync.dma_start(out=aux[:], in_=aux_d[:])

            # h histories (bf16, transposed layout): full for h0/h1 (feed
            # downstream projections), 16-step rings for h2/hs.
            h0 = hpool.tile([128, 4 * T * b], BF16, tag="h0")
            h1 = hpool.tile([128, 4 * T * b], BF16, tag="h1")
            h2 = hpool.tile([128, 4 * GS * b], BF16, tag="h2")
            hs = hpool.tile([128, 4 * GS * b], BF16, tag="hs")
            hist = [h0, h1, h2, hs]
            hlen = [T, T, GS, GS]  # cols per k-tile (in steps)

            out_sb = hpool.tile([128, 4 * b], F32, tag="osb")

            def proj_thunks(c, g, P):
                """Thunk list: proj mms of chain c, steps [GS*g, GS*(g+1)),
                into psum tile P (first mm resets the bank)."""
                t0 = GS * g
                if c == 0:
                    src = xT
                else:
                    src = h0 if c != 2 else h1
                slen = T
                thunks = []
                for m in range(4):
                    for k in range(KIN[c]):
                        def th(m=m, k=k, first=(m == 0 and k == 0)):
                            nc.tensor.matmul(
                                P[:, m * 128 : (m + 1) * 128],
                                wmT[:, WM_BASE[c] + (k * 4 + m) * 128 :][:, :128],
                                src[:, k * slen * b + t0 * b :][:, : GS * b],
                                start=first,
                                stop=False,
                                skip_group_check=True,
                            )
                        thunks.append(th)
                return thunks

            def stt_bias(c, P):
                nc.vector.scalar_tensor_tensor(
                    P[:],
                    P[:],
                    1.0,
                    bT[:, c * 512 : (c + 1) * 512],
                    mybir.AluOpType.mult,
                    mybir.AluOpType.add,
                )

            def recur_act(c, g, j, P):
                """One serial step for chain c: accumulate W_hh @ h_{t-1} into
                P's step slice, then tanh the slice out to the h history."""
                t = GS * g + j
                hbuf = hist[c]
                L = hlen[c]
                if t > 0:
                    tp = (t - 1) % L
                    last = t == T - 1 or j == GS - 1
                    for k in range(4):
                        for m in range(4):
                            nc.tensor.matmul(
                                P[:, m * 128 + j * b :][:, :b],
                                whT[:, c * 2048 + (k * 4 + m) * 128 :][:, :128],
                                hbuf[:, k * L * b + tp * b :][:, :b],
                                start=False,
                                stop=last and k == 3 and m == 3,
                                skip_group_check=True,
                            )
                tc_ = t % L
                pin = P[:].rearrange("p (m t v) -> p m t v", m=4, t=GS)[:, :, j, :]
                hout = hbuf[:].rearrange("p (k t v) -> p k t v", k=4, t=L)[:, :, tc_, :]
                nc.scalar.activation(hout, pin, mybir.ActivationFunctionType.Tanh)

            # wavefront (deepened so next-iter projs read complete h groups):
            # c0 at group it, c1/skip at it-2, c2 at it-4. Proj mms for
            # iteration it+1 are interleaved into iteration it's j-loop so
            # their streams hide under the serial LDW-bound recurrence.
            LAG = {0: 0, 1: 2, 3: 2, 2: 4}

            def grp(c, it):
                g = it - LAG[c]
                return g if 0 <= g < NG else None

            NIT = NG + 4
            cur = [None, None, None, None]
            nxt = [None, None, None, None]

            def build_pending(it):
                """Allocate next-iter psum tiles, return proj thunk list."""
                pend = []
                for c in (0, 1, 3, 2):
                    g = grp(c, it)
                    if g is not None:
                        nxt[c] = ppool.tile([128, 512], F32, name=f"ps{c}_{it}", tag=f"ps{c}")
                        pend.extend(proj_thunks(c, g, nxt[c]))
                    else:
                        nxt[c] = None
                return pend

            # prologue: fill iteration 0's banks directly
            pend = build_pending(0)
            for th in pend:
                th()
            for c in range(4):
                if nxt[c] is not None:
                    stt_bias(c, nxt[c])
            for it in range(NIT):
                cur, nxt = nxt, [None, None, None, None]
                pend = build_pending(it + 1) if it + 1 < NIT else []
                for j in range(GS):
                    for c in (0, 1, 3, 2):
                        g = grp(c, it)
                        if g is not None:
                            recur_act(c, g, j, cur[c])
                    # interleave next-iteration proj mms
                    lo = (j * len(pend)) // GS
                    hi = ((j + 1) * len(pend)) // GS
                    for th in pend[lo:hi]:
                        th()
                for c in range(4):
                    if nxt[c] is not None:
                        stt_bias(c, nxt[c])

            # out = h2[T-1] + hs[T-1]
            tf = (T - 1) % GS
            h2v = h2[:].rearrange("p (k t v) -> p k t v", k=4, t=GS)[:, :, tf, :]
            hsv = hs[:].rearrange("p (k t v) -> p k t v", k=4, t=GS)[:, :, tf, :]
            ov = out_sb[:].rearrange("p (k v) -> p k v", k=4)
            nc.vector.scalar_tensor_tensor(
                ov, h2v, 1.0, hsv, mybir.AluOpType.mult, mybir.AluOpType.add
            )
            nc.sync.dma_start(out=out_d[:], in_=out_sb[:])
    nc.finalize()
    return nc


def prep_inputs(x, Ws, T, b):
    """Per-core input dicts. Ws = dict of weight arrays (full precision)."""
    wm_list, wh_list, bias_list = [], [], []
    for c, suf in enumerate(["0", "1", "2", "s"]):
        wih = Ws[f"W_ih{suf}"]
        if f"mask{suf}" in Ws:
            wih = wih * Ws[f"mask{suf}"]
        wm_list.append(_tileize(np.ascontiguousarray(wih.T)))
        wh_list.append(_tileize(np.ascontiguousarray(Ws[f"W_hh{suf}"].T)))
        bias_list.append(Ws[f"b_ih{suf}"] + Ws[f"b_hh{suf}"])
    wmT = np.concatenate(wm_list, axis=1).astype(np.float32)
    assert wmT.shape[1] == WM_COLS
    whT = np.concatenate(wh_list, axis=1).astype(np.float32)
    aux = np.zeros((1, 2560), np.float32)
    aux[0, :512] = 1.0
    for c in range(4):
        aux[0, 512 + c * 512 : 512 + (c + 1) * 512] = bias_list[c]

    def bf16(a):
        import ml_dtypes

        return np.asarray(a).astype(ml_dtypes.bfloat16)

    bts = []
    for c in range(4):
        bc = bias_list[c].reshape(4, 128).T.astype(np.float32)  # [128, 4]
        bts.append(np.broadcast_to(bc[:, :, None], (128, 4, 128)).reshape(128, 512))
    biasT = np.concatenate(bts, axis=1)
    wmT, whT, aux, biasT = bf16(wmT), bf16(whT), bf16(aux), bf16(biasT)
    in_maps = []
    for g in range(NCORES):
        xg = x[:T, g * b : (g + 1) * b, :]  # [T, b, IN]
        arr = xg.transpose(2, 0, 1).reshape(IN, T * b)  # [IN, T*b]
        xT = (
            arr.reshape(2, 128, T * b).transpose(1, 0, 2).reshape(128, 2 * T * b)
        ).astype(np.float32)
        in_maps.append(
            {"xT": bf16(xT), "wmT": wmT, "whT": whT, "aux": aux, "biasT": biasT}
        )
    return in_maps


_CACHED = {}


def run(inputs, trace=False):
    inputs = {k: np.asarray(v, np.float32) for k, v in inputs.items()}
    x = np.asarray(inputs["x"], np.float32)
    T, B = x.shape[0], x.shape[1]
    b = B // NCORES
    in_maps = prep_inputs(x, inputs, T, b)
    key = (T, b)
    if key not in _CACHED:
        _CACHED[key] = build(T, b)
    nc = _CACHED[key]
    res = bass_utils.run_bass_kernel_spmd(
        nc, in_maps, core_ids=list(range(NCORES)), trace=trace
    )
    outs = []
    for g in range(NCORES):
        o = res.results[g]["out"]  # [128, 4*b]
        o = o.reshape(128, 4, b).transpose(1, 0, 2).reshape(H, b).T  # [b, H]
        outs.append(o)
    return np.concatenate(outs, axis=0).astype(np.float32), res  # [B, H]


def kernel(**inputs):
    return run(inputs, trace=False)[0]



# revision 4
# speedup vs baseline: 1.1856x; 1.1856x over previous
"""Trainium2 Bass kernel for the 4-chain masked RNN (ArbitraryStructureRNN).

Structure: out = last step of (layer2 + skip), where
  layer0: x -> h0 (RNN tanh, W_ih0 unmasked)
  layer1: h0 -> h1 (masked W_ih1)
  layer2: h1 -> h2 (masked W_ih2)
  skip:   h0 -> hs (masked W_ihs), added to layer2 output at the end.

Sharding: data-parallel over batch (B=64 -> 8 cores x 8).
Per core all 4 chains run as a wavefront (chain c lags its input producer
by one 16-step group). Input projections for a 16-step group are matmul'd
into the PSUM bank first (sets has_written), the serial recurrence matmuls
then accumulate on top (start=False) so the per-step add is free; tanh is
a single strided ACT per chain-step reading PSUM and writing the bf16
h-history in SBUF, which feeds both the next step's matmuls and the
downstream chain's projections.
"""

import sys, types

for _p in ("/opt/trn_rl_repo",):
    if _p not in sys.path:
        sys.path.append(_p)

import numpy as np

# make run_bass_kernel_spmd(trace=True) usable under axon (optional here)
try:
    import antenv

    if not hasattr(antenv, "axon_hooks"):
        _hooks = types.ModuleType("antenv.axon_hooks")
        _h = [None]
        _hooks.set_axon_ntff_profile_hook = lambda h: _h.__setitem__(0, h)
        _hooks.get_axon_ntff_profile_hook = lambda: _h[0]
        sys.modules["antenv.axon_hooks"] = _hooks
        antenv.axon_hooks = _hooks
        try:
            from trn_agent_boot.trn_boot import _ntff_profile_via_ctypes

            _hooks.set_axon_ntff_profile_hook(
                _ntff_profile_via_ctypes("/opt/axon/libaxon_pjrt.so")
            )
        except Exception:
            pass
except Exception:
    pass

from concourse import bacc, tile
from concourse import bass_utils
from concourse.bass import mybir

BF16 = mybir.dt.bfloat16
F32 = mybir.dt.float32

H = 512
IN = 256
NCORES = 8
GS = 16  # steps per psum bank group

# chain input-feature k-tile counts: c0 reads x (256 = 2 tiles), others read h (4)
KIN = [2, 4, 4, 4]
WM_BASE = [0, 1024, 3072, 5120]  # col base of each chain in wmT
WM_COLS = WM_BASE[-1] + 2048


def _tileize(a):
    """[K, M] -> [128, (K/128)*(M/128)*128] with col = (k*nm + m)*128 + j."""
    K, M = a.shape
    nk, nm = K // 128, M // 128
    return (
        a.reshape(nk, 128, nm, 128).transpose(1, 0, 2, 3).reshape(128, nk * nm * 128)
    )


def build(T, b):
    NG = T // GS
    nc = bacc.Bacc("TRN2", target_bir_lowering=False, debug=False, num_devices=NCORES)
    xT_d = nc.dram_tensor("xT", [128, 2 * T * b], BF16, kind="ExternalInput").ap()
    wmT_d = nc.dram_tensor("wmT", [128, WM_COLS], BF16, kind="ExternalInput").ap()
    whT_d = nc.dram_tensor("whT", [128, 8192], BF16, kind="ExternalInput").ap()
    aux_d = nc.dram_tensor("aux", [1, 2560], BF16, kind="ExternalInput").ap()
    bT_d = nc.dram_tensor("biasT", [128, 2048], BF16, kind="ExternalInput").ap()
    out_d = nc.dram_tensor("out", [128, 4 * b], F32, kind="ExternalOutput").ap()

    with tile.TileContext(nc) as tc:
        with (
            tc.tile_pool(name="const", bufs=1) as cpool,
            tc.tile_pool(name="hist", bufs=1) as hpool,
            tc.tile_pool(name="ps", bufs=2, space="PSUM") as ppool,
        ):
            xT = cpool.tile([128, 2 * T * b], BF16, tag="xT")
            wmT = cpool.tile([128, WM_COLS], BF16, tag="wmT")
            whT = cpool.tile([128, 8192], BF16, tag="whT")
            aux = cpool.tile([1, 2560], BF16, tag="aux")
            bT = cpool.tile([128, 2048], BF16, tag="bT")
            nc.sync.dma_start(out=xT[:], in_=xT_d[:])
            nc.sync.dma_start(out=wmT[:], in_=wmT_d[:])
            nc.sync.dma_start(out=whT[:], in_=whT_d[:])
            nc.sync.dma_start(out=aux[:], in_=aux_d[:])
            nc.sync.dma_start(out=bT[:], in_=bT_d[:])

            # h histories (bf16, transposed layout): full for h0/h1 (feed
            # downstream projections), 16-step rings for h2/hs.
            h0 = hpool.tile([128, 4 * T * b], BF16, tag="h0")
            h1 = hpool.tile([128, 4 * T * b], BF16, tag="h1")
            h2 = hpool.tile([128, 4 * GS * b], BF16, tag="h2")
            hs = hpool.tile([128, 4 * GS * b], BF16, tag="hs")
            hist = [h0, h1, h2, hs]
            hlen = [T, T, GS, GS]  # cols per k-tile (in steps)

            out_sb = hpool.tile([128, 4 * b], F32, tag="osb")

            def proj_thunks(c, g, P):
                """Thunk list: proj mms of chain c, steps [GS*g, GS*(g+1)),
                into psum tile P (first mm resets the bank)."""
                t0 = GS * g
                if c == 0:
                    src = xT
                else:
                    src = h0 if c != 2 else h1
                slen = T
                thunks = []
                for m in range(4):
                    for k in range(KIN[c]):
                        def th(m=m, k=k, first=(m == 0 and k == 0)):
                            nc.tensor.matmul(
                                P[:, m * 128 : (m + 1) * 128],
                                wmT[:, WM_BASE[c] + (k * 4 + m) * 128 :][:, :128],
                                src[:, k * slen * b + t0 * b :][:, : GS * b],
                                start=first,
                                stop=False,
                                skip_group_check=True,
                            )
                        thunks.append(th)
                return thunks

            def stt_bias(c, P):
                nc.vector.scalar_tensor_tensor(
                    P[:],
                    P[:],
                    1.0,
                    bT[:, c * 512 : (c + 1) * 512],
                    mybir.AluOpType.mult,
                    mybir.AluOpType.add,
                )

            def recur_act(c, g, j, P):
                """One serial step for chain c: accumulate W_hh @ h_{t-1} into
                P's step slice, then tanh the slice out to the h history."""
                t = GS * g + j
                hbuf = hist[c]
                L = hlen[c]
                if t > 0:
                    tp = (t - 1) % L
                    last = t == T - 1 or j == GS - 1
                    for k in range(4):
                        for m in range(4):
                            nc.tensor.matmul(
                                P[:, m * 128 + j * b :][:, :b],
                                whT[:, c * 2048 + (k * 4 + m) * 128 :][:, :128],
                                hbuf[:, k * L * b + tp * b :][:, :b],
                                start=False,
                                stop=last and k == 3 and m == 3,
                                skip_group_check=True,
                            )
                tc_ = t % L
                pin = P[:].rearrange("p (m t v) -> p m t v", m=4, t=GS)[:, :, j, :]
                hout = hbuf[:].rearrange("p (k t v) -> p k t v", k=4, t=L)[:, :, tc_, :]
                nc.scalar.activation(hout, pin, mybir.ActivationFunctionType.Tanh)

            # wavefront (deepened so next-iter projs read complete h groups):
            # c0 at group it, c1/skip at it-2, c2 at it-4. Proj mms for
            # iteration it+1 are interleaved into iteration it's j-loop so
            # their streams hide under the serial LDW-bound recurrence.
            LAG = {0: 0, 1: 2, 3: 2, 2: 4}

            def grp(c, it):
                g = it - LAG[c]
                return g if 0 <= g < NG else None

            NIT = NG + 4
            cur = [None, None, None, None]
            nxt = [None, None, None, None]

            def build_pending(it):
                """Allocate next-iter psum tiles, return proj thunk list."""
                pend = []
                for c in (0, 1, 3, 2):
                    g = grp(c, it)
                    if g is not None:
                        nxt[c] = ppool.tile([128, 512], F32, name=f"ps{c}_{it}", tag=f"ps{c}")
                        pend.extend(proj_thunks(c, g, nxt[c]))
                    else:
                        nxt[c] = None
                return pend

            # prologue: fill iteration 0's banks directly
            pend = build_pending(0)
            for th in pend:
                th()
            for c in range(4):
                if nxt[c] is not None:
                    stt_bias(c, nxt[c])
            for it in range(NIT):
                cur, nxt = nxt, [None, None, None, None]
                pend = build_pending(it + 1) if it + 1 < NIT else []
                for j in range(GS):
                    for c in (0, 1, 3, 2):
                        g = grp(c, it)
                        if g is not None:
                            recur_act(c, g, j, cur[c])
                    # interleave next-iteration proj mms
                    lo = (j * len(pend)) // GS
                    hi = ((j + 1) * len(pend)) // GS
                    for th in pend[lo:hi]:
                        th()
                for c in range(4):
                    if nxt[c] is not None:
                        stt_bias(c, nxt[c])

            # out = h2[T-1] + hs[T-1]
            tf = (T - 1) % GS
            h2v = h2[:].rearrange("p (k t v) -> p k t v", k=4, t=GS)[:, :, tf, :]
            hsv = hs[:].rearrange("p (k t v) -> p k t v", k=4, t=GS)[:, :, tf, :]
            ov = out_sb[:].rearrange("p (k v) -> p k v", k=4)
            nc.vector.scalar_tensor_tensor(
                ov, h2v, 1.0, hsv, mybir.AluOpType.mult, mybir.AluOpType.add
            )
            nc.sync.dma_start(out=out_d[:], in_=out_sb[:])
    nc.finalize()
    return nc


def prep_inputs(x, Ws, T, b):
    """Per-core input dicts. Ws = dict of weight arrays (full precision)."""
    wm_list, wh_list, bias_list = [], [], []
    for c, suf in enumerate(["0", "1", "2", "s"]):
        wih = Ws[f"W_ih{suf}"]
        if f"mask{suf}" in Ws:
            wih = wih * Ws[f"mask{suf}"]
        wm_list.append(_tileize(np.ascontiguousarray(wih.T)))
        wh_list.append(_tileize(np.ascontiguousarray(Ws[f"W_hh{suf}"].T)))
        bias_list.append(Ws[f"b_ih{suf}"] + Ws[f"b_hh{suf}"])
    wmT = np.concatenate(wm_list, axis=1).astype(np.float32)
    assert wmT.shape[1] == WM_COLS
    whT = np.concatenate(wh_list, axis=1).astype(np.float32)
    aux = np.zeros((1, 2560), np.float32)
    aux[0, :512] = 1.0
    for c in range(4):
        aux[0, 512 + c * 512 : 512 + (c + 1) * 512] = bias_list[c]

    def bf16(a):
        import ml_dtypes

        return np.asarray(a).astype(ml_dtypes.bfloat16)

    bts = []
    for c in range(4):
        bc = bias_list[c].reshape(4, 128).T.astype(np.float32)  # [128, 4]
        bts.append(np.broadcast_to(bc[:, :, None], (128, 4, 128)).reshape(128, 512))
    biasT = np.concatenate(bts, axis=1)
    wmT, whT, aux, biasT = bf16(wmT), bf16(whT), bf16(aux), bf16(biasT)
    in_maps = []
    for g in range(NCORES):
        xg = x[:T, g * b : (g + 1) * b, :]  # [T, b, IN]
        arr = xg.transpose(2, 0, 1).reshape(IN, T * b)  # [IN, T*b]
        xT = (
            arr.reshape(2, 128, T * b).transpose(1, 0, 2).reshape(128, 2 * T * b)
        ).astype(np.float32)
        in_maps.append(
            {"xT": bf16(xT), "wmT": wmT, "whT": whT, "aux": aux, "biasT": biasT}
        )
    return in_maps


_CACHED = {}


def run(inputs, trace=False):
    inputs = {k: np.asarray(v, np.float32) for k, v in inputs.items()}
    x = np.asarray(inputs["x"], np.float32)
    T, B = x.shape[0], x.shape[1]
    b = B // NCORES
    in_maps = prep_inputs(x, inputs, T, b)
    key = (T, b)
    if key not in _CACHED:
        _CACHED[key] = build(T, b)
    nc = _CACHED[key]
    res = bass_utils.run_bass_kernel_spmd(
        nc, in_maps, core_ids=list(range(NCORES)), trace=trace
    )
    outs = []
    for g in range(NCORES):
        o = res.results[g]["out"]  # [128, 4*b]
        o = o.reshape(128, 4, b).transpose(1, 0, 2).reshape(H, b).T  # [b, H]
        outs.append(o)
    return np.concatenate(outs, axis=0).astype(np.float32), res  # [B, H]


def kernel(**inputs):
    return run(inputs, trace=False)[0]

